# revision 1
# baseline (speedup 1.0000x reference)
"""Trainium2 Bass kernel for single-head attention with RoPE.

Problem (per full input): x [256, 200, 1024], wq/wk/wv [128, 1024], wo [1024, 128]
  q/k/v = x @ w*.T ; RoPE on q,k (positions 1..S-1, class token 0 unrotated)
  out = softmax(q k^T / sqrt(128)) v @ wo.T

Strategy: data-parallel over batch across 8 NeuronCores (32 batches/core).
Per core, per block of 2 batches (400 tokens):
  - x is pre-transposed on host to d-major [dc, 128, t] so QKV projections are
    natural matmuls (contraction dim on partitions).
  - q,k computed head-major [128h, t] with de-interleaved RoPE (wq/wk rows are
    permuted host-side so rotate-half form applies to contiguous 64-row halves);
    RoPE = 2 tensor_mul + tensor_sub/tensor_add on DVE using host cos/sin tables.
  - v head-major, then PE-transposed to seq-major [k, h] for the AV matmul.
  - scores computed in BOTH orientations ([q,k] for softmax row-sums via the
    ACT engine's accum_out, [k,q] for the AV matmul) — cheaper than transposing
    the softmax output. No max-subtraction (fp32 exp is safe at these scales).
  - softmax normalization (1/rowsum) is folded into the final PSUM->SBUF copy
    of the output projection as a per-partition activation scale.
"""

import math

import numpy as np

import concourse.bass as bass
import concourse.mybir as mybir
import concourse.tile as tile
from concourse.bass_utils import run_bass_kernel_spmd

B, S, DIM, HD = 256, 200, 1024, 128
BASE = 10000.0
N_CORES = 8
BS = B // N_CORES      # 32 batches per core
BB = 2                 # batches per block
TB = BB * S            # 400 tokens per block
NBLK = BS // BB        # 16 blocks per core
NDC = DIM // 128       # 8 contraction chunks
F32 = mybir.dt.float32
F32R = mybir.dt.float32r
EXP_SCALE = 1.0 / math.sqrt(HD)
# float32r: fp32-layout operands consumed by the PE's single-pass reduced
# precision matmul (~1.8e-4 rel err at K=128; 4x faster than true fp32 for
# moving dim >= 256). Every matmul uses it; score/AV matmuls pad their
# moving dim from S=200 to SP=256 to stay on the fast path.
# q-position (= output row) chunks within one batch: [0:128], [128:200]
QCH = [(0, 128), (128, S - 128)]
# score/AV matmuls run with moving dim padded to 256 (f32r is 4x slower
# below N=256); the pad columns are zeros / never-read garbage
SP = 256


class _TileContextSplitDrain(tile.TileContext):
    """Workaround: this walrus build rejects >2 sem-wait commands on the
    kernel-tail Drain. Emit each needed wait as its own instruction first."""

    def _drain_and_barrier(self, tick_clock, wait_clock):
        nc = self.nc
        fake = mybir.InstNoOp(
            name=nc.get_next_instruction_name(), ins=[], outs=[],
            engine=mybir.EngineType.SP,
        )
        wait_clock.add_sem_waits(
            fake, tile.ScopedClock({None: tick_clock.global_clock})
        )
        waits = list(fake.sync_info.on_wait) if fake.sync_info is not None else []
        assert self.sems is not None
        handles = {h.name: h for h in self.sems.allocated().values()}
        for w in waits:
            nc.sync.wait_ge(handles[w.ant_name], w.wait_value)
        nc.sync.drain()
        nc.all_engine_barrier()
        popped = nc._tile_sem_poison_stack.pop()
        assert popped is self._sem_poison
        nc.clear_and_free_semaphores(list(self.sems.allocated().values()))
        nc.all_engine_barrier()


def _split_excess_waits(nc):
    """This walrus build accepts 1 sem-wait per instruction (2 on
    EventSemaphore). Tile may attach more; hoist the excess onto standalone
    EventSemaphore instructions right before the owner (same engine, so
    in-order issue preserves the wait semantics)."""
    n = 0
    for b in nc.m.functions[0].blocks:
        insts = b.instructions
        out = []
        for i in insts:
            si = i.sync_info
            if si is not None and len(si.on_wait) > 1:
                keep = 2 if isinstance(i, mybir.InstEventSemaphore) else 1
                waits = list(si.on_wait)
                for w in waits[:-keep] if keep < len(waits) else []:
                    n += 1
                    out.append(mybir.InstEventSemaphore(
                        name=f"{i.name}-evw{n}", ins=[], outs=[],
                        engine=i.engine,
                        sync_info=mybir.SyncInfo(on_wait=[w], on_update=[]),
                    ))
                i.sync_info = mybir.SyncInfo(
                    on_wait=waits[-keep:], on_update=list(si.on_update)
                )
            out.append(i)
        b.instructions = out
    return n


def _build_nc():
    nc = bass.Bass("TRN2", target_bir_lowering=False, debug=False)

    xt = nc.dram_tensor("xt", [NBLK, 128, NDC, TB], F32R, kind="ExternalInput").ap()
    wqt = nc.dram_tensor("wqt", [128, NDC, HD], F32R, kind="ExternalInput").ap()
    wkt = nc.dram_tensor("wkt", [128, NDC, HD], F32R, kind="ExternalInput").ap()
    wvt = nc.dram_tensor("wvt", [128, NDC, HD], F32R, kind="ExternalInput").ap()
    wot = nc.dram_tensor("wot", [HD, DIM], F32R, kind="ExternalInput").ap()
    cosf = nc.dram_tensor("cosf", [128, TB], F32, kind="ExternalInput").ap()
    sinf = nc.dram_tensor("sinf", [128, TB], F32, kind="ExternalInput").ap()
    ident = nc.dram_tensor("ident", [128, 128], F32R, kind="ExternalInput").ap()
    p64 = nc.dram_tensor("p64", [128, 128], F32R, kind="ExternalInput").ap()
    out = nc.dram_tensor("out", [BS, S, DIM], F32, kind="ExternalOutput").ap()

    with _TileContextSplitDrain(nc) as tc:
        with (
            tc.tile_pool(name="singles", bufs=1) as singles,
            tc.tile_pool(name="xt", bufs=4) as xt_pool,
            tc.tile_pool(name="qkv_ps", bufs=2, space="PSUM") as qkv_ps,
            tc.tile_pool(name="attn_ps", bufs=3, space="PSUM") as attn_ps,
            tc.tile_pool(name="out_ps", bufs=3, space="PSUM") as out_ps,
            tc.tile_pool(name="ropetmp", bufs=2) as ropetmp,
            tc.tile_pool(name="heads", bufs=3) as heads,
            tc.tile_pool(name="attn_sb", bufs=4) as attn_sb_pool,
            tc.tile_pool(name="stats", bufs=12) as stats,
            tc.tile_pool(name="outsb", bufs=3) as outsb,
        ):
            # ---- one-time loads (wq + first x chunks first: they gate the
            # first projection matmuls; everything else can land behind) ----
            w_sb = {}
            t = singles.tile([128, NDC * HD], F32R, name="wq", tag="wq")
            nc.sync.dma_start(out=t, in_=wqt)
            w_sb["wq"] = t
            xt0_sb = xt_pool.tile([128, NDC * TB], F32R, name="xt", tag="xt")
            for dc in range(NDC):
                nc.sync.dma_start(
                    out=xt0_sb[:, dc * TB:(dc + 1) * TB], in_=xt[0, :, dc, :]
                )
            for name, src in (("wk", wkt), ("wv", wvt)):
                t = singles.tile([128, NDC * HD], F32R, name=name, tag=name)
                nc.sync.dma_start(out=t, in_=src)
                w_sb[name] = t
            p64_sb = singles.tile([128, 128], F32R, name="p64", tag="p64")
            nc.sync.dma_start(out=p64_sb, in_=p64)
            cos_sb = singles.tile([128, TB], F32, name="cosf", tag="cosf")
            nc.sync.dma_start(out=cos_sb, in_=cosf)
            sin_sb = singles.tile([128, TB], F32, name="sinf", tag="sinf")
            nc.sync.dma_start(out=sin_sb, in_=sinf)
            id_sb = singles.tile([128, 128], F32R, name="ident", tag="ident")
            nc.sync.dma_start(out=id_sb, in_=ident)
            wot_sb = singles.tile([HD, DIM], F32R, name="wot", tag="wot")
            nc.sync.dma_start(out=wot_sb, in_=wot)

            for blk in range(NBLK):
                # ---- load x^T for this block ----
                if blk == 0:
                    xt_sb = xt0_sb
                else:
                    xt_sb = xt_pool.tile([128, NDC * TB], F32R, name="xt",
                                         tag="xt")
                    nc.sync.dma_start(out=xt_sb, in_=xt[blk])

                # ---- QKV projections (head-major [128h, TB]) ----
                def proj(wname):
                    ps = qkv_ps.tile([128, TB], F32, name="proj_ps", tag="proj_ps")
                    for dc in range(NDC):
                        nc.tensor.matmul(
                            ps,
                            lhsT=w_sb[wname][:, dc * HD:(dc + 1) * HD],
                            rhs=xt_sb[:, dc * TB:(dc + 1) * TB],
                            start=(dc == 0),
                            stop=(dc == NDC - 1),
                        )
                    return ps

                q_ps = proj("wq")
                k_ps = proj("wk")
                v_ps = proj("wv")

                # ---- RoPE (de-interleaved rotate-half form) ----
                # DVE lanes cannot mix base partitions, so the half-swap
                # swap(q)[p] = q[(p+64)%128] runs on the PE via a permutation
                # matmul (rhs must be SBUF, hence the ACT evacuation first).
                def rope(ps, tag):
                    qsb = ropetmp.tile([128, TB], F32R, name="pre_" + tag,
                                       tag="pre_" + tag)
                    nc.scalar.copy(qsb, ps)
                    sw_ps = qkv_ps.tile([128, TB], F32, name="proj_ps",
                                        tag="proj_ps")
                    nc.tensor.matmul(sw_ps, lhsT=p64_sb, rhs=qsb,
                                     start=True, stop=True)
                    c = ropetmp.tile([128, TB], F32, name="rope_c", tag="rope_c")
                    u = ropetmp.tile([128, TB], F32, name="rope_u", tag="rope_u")
                    h = heads.tile([128, 2 * SP], F32R, name=tag, tag=tag)
                    nc.gpsimd.memset(h[:, TB:].bitcast(F32), 0.0)
                    nc.vector.tensor_mul(c, qsb.bitcast(F32), cos_sb)
                    # sin table is sign-folded ([-sin; +sin]) so one add
                    # completes the rotation
                    nc.vector.tensor_mul(u, sw_ps, sin_sb)
                    nc.vector.tensor_add(h[:, 0:TB], c, u)
                    return h

                q_h = rope(q_ps, "q_h")
                k_h = rope(k_ps, "k_h")
                v_h = heads.tile([128, TB], F32R, name="v_h", tag="v_h")
                nc.scalar.copy(v_h, v_ps)

                # ---- scores, both orientations, per batch ----
                # s[qc]: [q, k] for row sums; st[kc]: [k, q] for the AV matmul
                s_tiles = []
                for qc, (q0, qsz) in enumerate(QCH):
                    sp = attn_ps.tile([128, 2 * SP], F32, name="attn_ps",
                                      tag="attn_ps")
                    for i in range(BB):
                        nc.tensor.matmul(
                            sp[0:qsz, i * SP:(i + 1) * SP],
                            lhsT=q_h[:, i * S + q0: i * S + q0 + qsz],
                            rhs=k_h[:, i * S: i * S + SP],
                            start=True, stop=True,
                        )
                    s_tiles.append(sp)

                # exp row sums via ACT accum_out (per batch, per q-chunk)
                recips = {}
                scratch = [
                    attn_sb_pool.tile([128, 2 * SP], F32, name="exp_scr", tag="exp_scr")
                    for _ in QCH
                ]
                for qc, (q0, qsz) in enumerate(QCH):
                    for i in range(BB):
                        sums = stats.tile([128, 1], F32, name="sums", tag="sums")
                        nc.scalar.activation(
                            out=scratch[qc][0:qsz, i * SP: i * SP + S],
                            in_=s_tiles[qc][0:qsz, i * SP: i * SP + S],
                            func=mybir.ActivationFunctionType.Exp,
                            scale=EXP_SCALE,
                            accum_out=sums[0:qsz, :],
                        )
                        rec = stats.tile([128, 1], F32, name="recip", tag="recip")
                        nc.vector.reciprocal(rec[0:qsz, :], sums[0:qsz, :])
                        recips[(i, qc)] = rec

                est = []
                for kc, (k0, ksz) in enumerate(QCH):
                    stp = attn_ps.tile([128, 2 * SP], F32, name="attn_ps",
                                       tag="attn_ps")
                    for i in range(BB):
                        nc.tensor.matmul(
                            stp[0:ksz, i * SP:(i + 1) * SP],
                            lhsT=k_h[:, i * S + k0: i * S + k0 + ksz],
                            rhs=q_h[:, i * S: i * S + SP],
                            start=True, stop=True,
                        )
                    e = attn_sb_pool.tile([128, 2 * SP], F32R, name="exp_st",
                                          tag="exp_st")
                    nc.scalar.activation(
                        out=e[0:ksz, :], in_=stp[0:ksz, :],
                        func=mybir.ActivationFunctionType.Exp,
                        scale=EXP_SCALE,
                    )
                    est.append(e)

                # ---- V -> seq-major via PE transpose, per batch ----
                vt_sbs = []
                for i in range(BB):
                    vt_ps = attn_ps.tile([128, 256], F32R, name="vt_ps", tag="attn_ps")
                    nc.tensor.transpose(
                        vt_ps[0:128, 0:128],
                        v_h[:, i * S: i * S + 128], id_sb,
                    )
                    nc.tensor.transpose(
                        vt_ps[0:72, 128:256],
                        v_h[:, i * S + 128: (i + 1) * S], id_sb,
                    )
                    vt_sb = attn_sb_pool.tile([128, 256], F32R, name="vt_sb", tag="vt_sb")
                    nc.vector.tensor_copy(
                        vt_sb[0:128, 0:128], vt_ps[0:128, 0:128]
                    )
                    nc.vector.tensor_copy(
                        vt_sb[0:72, 128:256], vt_ps[0:72, 128:256]
                    )
                    vt_sbs.append(vt_sb)

                # ---- AV: attn_head[h, q] (unnormalized) ----
                av_ps = attn_ps.tile([128, 2 * SP], F32, name="attn_ps",
                                     tag="attn_ps")
                for i in range(BB):
                    nc.tensor.matmul(
                        av_ps[:, i * SP:(i + 1) * SP],
                        lhsT=vt_sbs[i][0:128, 0:128],
                        rhs=est[0][0:128, i * SP:(i + 1) * SP],
                        start=True, stop=False,
                    )
                    nc.tensor.matmul(
                        av_ps[:, i * SP:(i + 1) * SP],
                        lhsT=vt_sbs[i][0:72, 128:256],
                        rhs=est[1][0:72, i * SP:(i + 1) * SP],
                        start=False, stop=True,
                    )
                av_sb = attn_sb_pool.tile([128, 2 * SP], F32R, name="av_sb",
                                          tag="av_sb")
                for i in range(BB):
                    nc.vector.tensor_copy(
                        av_sb[:, i * SP:(i + 1) * SP],
                        av_ps[:, i * SP:(i + 1) * SP],
                    )

                # ---- output projection + normalization + store ----
                osb = {
                    (i, tc_i): outsb.tile([128, DIM], F32,
                                          name=f"osb{i}_{tc_i}",
                                          tag=f"osb{i}_{tc_i}")
                    for i in range(BB) for tc_i in range(len(QCH))
                }
                for i in range(BB):
                    for tc_i, (t0, tsz) in enumerate(QCH):
                        for dc in range(2):
                            ops = out_ps.tile([128, 512], F32, name="out_ps", tag="out_ps")
                            nc.tensor.matmul(
                                ops[0:tsz, :],
                                lhsT=av_sb[:, i * SP + t0: i * SP + t0 + tsz],
                                rhs=wot_sb[:, dc * 512:(dc + 1) * 512],
                                start=True, stop=True,
                            )
                            dst = osb[(i, tc_i)][0:tsz, dc * 512:
                                                 (dc + 1) * 512]
                            if dc == 0:
                                nc.scalar.mul(dst, ops[0:tsz, :],
                                              recips[(i, tc_i)][0:tsz, :])
                            else:
                                nc.vector.tensor_scalar_mul(
                                    dst, ops[0:tsz, :],
                                    recips[(i, tc_i)][0:tsz, :])
                for i in range(BB):
                    for tc_i, (t0, tsz) in enumerate(QCH):
                        nc.sync.dma_start(
                            out=out[blk * BB + i, t0:t0 + tsz, :],
                            in_=osb[(i, tc_i)][0:tsz, :],
                        )
    _split_excess_waits(nc)
    return nc


_NC_CACHE = {}


def _get_nc():
    if "nc" not in _NC_CACHE:
        _NC_CACHE["nc"] = _build_nc()
    return _NC_CACHE["nc"]


def _host_prep(x, wq, wk, wv, wo):
    """Shared (non-x) device inputs + per-core x^T shards."""
    perm = np.concatenate([np.arange(0, HD, 2), np.arange(1, HD, 2)])
    # weight layout [p, dc, h]: row d of w.T at (p=d%128, dc=d//128)
    wqt = np.ascontiguousarray(
        wq[perm].T.reshape(NDC, 128, HD).transpose(1, 0, 2))
    wkt = np.ascontiguousarray(
        wk[perm].T.reshape(NDC, 128, HD).transpose(1, 0, 2))
    wvt = np.ascontiguousarray(
        wv.T.reshape(NDC, 128, HD).transpose(1, 0, 2))
    wot = np.ascontiguousarray(wo.T)

    inv_freq = 1.0 / BASE ** (np.arange(0, HD, 2, dtype=np.float64) / HD)
    ang = np.zeros((S, HD // 2), np.float64)
    ang[1:] = np.arange(S - 1, dtype=np.float64)[:, None] * inv_freq[None, :]
    cos_t = np.cos(ang).T.astype(np.float32)   # [64, S]
    sin_t = np.sin(ang).T.astype(np.float32)
    cosf = np.tile(np.concatenate([cos_t, cos_t], axis=0), (1, BB))  # [128, TB]
    # sign-folded: rotated = q*cosf + swap64(q)*sinf in one add
    sinf = np.tile(np.concatenate([-sin_t, sin_t], axis=0), (1, BB))
    ident = np.eye(128, dtype=np.float32)

    shared = {
        "wqt": wqt, "wkt": wkt, "wvt": wvt, "wot": wot,
        "cosf": np.ascontiguousarray(cosf),
        "sinf": np.ascontiguousarray(sinf),
        "ident": ident,
        "p64": np.ascontiguousarray(
            np.roll(np.eye(128, dtype=np.float32), 64, axis=1)
        ),
    }
    xts = []
    for c in range(N_CORES):
        xc = x[c * BS:(c + 1) * BS].reshape(NBLK, TB, NDC, 128)
        # [blk, p, dc, t]: one contiguous 12.8KB run per partition per DMA
        xt = np.ascontiguousarray(xc.transpose(0, 3, 2, 1))
        xts.append(xt)
    return shared, xts


def kernel(x, wq, wk, wv, wo):
    x = np.asarray(x, np.float32)
    wq = np.asarray(wq, np.float32)
    wk = np.asarray(wk, np.float32)
    wv = np.asarray(wv, np.float32)
    wo = np.asarray(wo, np.float32)

    shared, xts = _host_prep(x, wq, wk, wv, wo)
    in_maps = [dict(shared, xt=xts[c]) for c in range(N_CORES)]
    nc = _get_nc()
    res = run_bass_kernel_spmd(nc, in_maps, list(range(N_CORES)))
    return np.concatenate(
        [res.results[c]["out"] for c in range(N_CORES)], axis=0
    ).reshape(B, S, DIM)



# revision 35
# speedup vs baseline: 1.5312x; 1.5312x over previous
"""Trainium2 Bass kernel for single-head attention with RoPE (fp16 pipeline).

Problem (per full input): x [256, 200, 1024], wq/wk/wv [128, 1024], wo [1024, 128]
  q/k/v = x @ w*.T ; RoPE on q,k (positions 1..S-1, class token 0 unrotated)
  out = softmax(q k^T / sqrt(128)) v @ wo.T

Strategy: data-parallel over batch across 8 NeuronCores (32 batches/core).
All device data is fp16 (rel-err budget 2e-2; fp16 keeps us ~1e-3):
  - halves DMA bytes (DMA was the top-busy device at fp32)
  - matmuls run 1 cycle/row at any moving size (no fp32r >=256 constraint)
  - DVE elementwise ops hit the 2-byte fast path

Per core, per block of 2 batches (400 tokens):
  - x pre-transposed on host to d-major [128, dc, t]; QKV projections are
    plain matmuls with contraction on partitions.
  - RoPE rotate-half swap done with two partition-shifted DVE copies
    (lane i of op reads partition base+i; bases differ per operand), no PE.
  - scores computed ONCE in [k, q] orientation; exp on ACT evacuates to
    SBUF fp16; row sums come free as est^T @ ones matmuls (moving dim 1).
  - V transposed seq-major via DMA xbar transpose (fp16 16x128 tiles).
  - softmax 1/rowsum folded into the output-projection PSUM->SBUF evac
    as a per-partition scale (ACT/DVE/Pool round-robin).
  - deep software pipeline: iteration i runs QKV(i), scores(i-1),
    AV(i-2), out-proj(i-3) on the PE, so every PE instruction's deps
    are >= 1 iteration stale.  This matters because the cost model's
    PE p-state ramp resets whenever the PE run queue drains: matmuls
    dispatched right after a drain run at 0.65-1.2 GHz instead of
    2.4 GHz for the next ~3us.  Out-proj matmuls interleave into QKV
    so PSUM-bank evacuations never stall the PE either.
"""

import math

import numpy as np

import concourse.bass as bass
import concourse.mybir as mybir
import concourse.tile as tile
from concourse.bass_utils import run_bass_kernel_spmd

B, S, DIM, HD = 256, 200, 1024, 128
BASE = 10000.0
N_CORES = 8
BS = B // N_CORES      # 32 batches per core
BB = 2                 # batches per block
TB = BB * S            # 400 tokens per block
NBLK = BS // BB        # 16 blocks per core
NDC = DIM // 128       # 8 contraction chunks
CH = 100               # token chunk for out-proj (4 equal chunks per block)
NCH = TB // CH         # 4
VP = 256               # padded per-batch v columns (xbar needs %128)
F32 = mybir.dt.float32
F16 = mybir.dt.float16
EXP_SCALE = 1.0 / math.sqrt(HD)
# k-chunks within one batch: (offset, size)
KCH = [(0, 128), (128, S - 128)]


class _TileContextSplitDrain(tile.TileContext):
    """Workaround: this walrus build rejects >2 sem-wait commands on the
    kernel-tail Drain. Emit each needed wait as its own instruction first."""

    def _drain_and_barrier(self, tick_clock, wait_clock):
        nc = self.nc
        fake = mybir.InstNoOp(
            name=nc.get_next_instruction_name(), ins=[], outs=[],
            engine=mybir.EngineType.SP,
        )
        wait_clock.add_sem_waits(
            fake, tile.ScopedClock({None: tick_clock.global_clock})
        )
        waits = list(fake.sync_info.on_wait) if fake.sync_info is not None else []
        assert self.sems is not None
        handles = {h.name: h for h in self.sems.allocated().values()}
        for w in waits:
            nc.sync.wait_ge(handles[w.ant_name], w.wait_value)
        nc.sync.drain()
        nc.all_engine_barrier()
        popped = nc._tile_sem_poison_stack.pop()
        assert popped is self._sem_poison
        nc.clear_and_free_semaphores(list(self.sems.allocated().values()))
        nc.all_engine_barrier()


def _split_excess_waits(nc):
    """This walrus build accepts 1 sem-wait per instruction (2 on
    EventSemaphore). Tile may attach more; hoist the excess onto standalone
    EventSemaphore instructions right before the owner (same engine, so
    in-order issue preserves the wait semantics)."""
    n = 0
    for b in nc.m.functions[0].blocks:
        insts = b.instructions
        out = []
        for i in insts:
            si = i.sync_info
            if si is not None and len(si.on_wait) > 1:
                keep = 2 if isinstance(i, mybir.InstEventSemaphore) else 1
                waits = list(si.on_wait)
                for w in waits[:-keep] if keep < len(waits) else []:
                    n += 1
                    out.append(mybir.InstEventSemaphore(
                        name=f"{i.name}-evw{n}", ins=[], outs=[],
                        engine=i.engine,
                        sync_info=mybir.SyncInfo(on_wait=[w], on_update=[]),
                    ))
                i.sync_info = mybir.SyncInfo(
                    on_wait=waits[-keep:], on_update=list(si.on_update)
                )
            out.append(i)
        b.instructions = out
    return n


STAGE_MARKS = []


def _build_nc():
    STAGE_MARKS.clear()
    nc = bass.Bass("TRN2", target_bir_lowering=False, debug=False)

    def mark(label):
        STAGE_MARKS.append((nc.next_id(), label))

    xt = nc.dram_tensor("xt", [NBLK, 128, NDC, TB], F16, kind="ExternalInput").ap()
    wqt = nc.dram_tensor("wqt", [128, NDC, HD], F16, kind="ExternalInput").ap()
    wkt = nc.dram_tensor("wkt", [128, NDC, HD], F16, kind="ExternalInput").ap()
    wvt = nc.dram_tensor("wvt", [128, NDC, HD], F16, kind="ExternalInput").ap()
    wot = nc.dram_tensor("wot", [HD, DIM], F16, kind="ExternalInput").ap()
    cosf = nc.dram_tensor("cosf", [128, TB], F16, kind="ExternalInput").ap()
    sinf = nc.dram_tensor("sinf", [128, TB], F16, kind="ExternalInput").ap()
    sin2f = nc.dram_tensor("sin2f", [128, TB], F16, kind="ExternalInput").ap()
    onesd = nc.dram_tensor("onesd", [128, 1], F16, kind="ExternalInput").ap()
    identd = nc.dram_tensor("identd", [128, 128], F16, kind="ExternalInput").ap()
    outd = nc.dram_tensor("outd", [NBLK, NCH, CH, DIM], F16,
                          kind="ExternalOutput").ap()

    with _TileContextSplitDrain(nc) as tc:
        with (
            tc.tile_pool(name="singles", bufs=1) as singles,
            tc.tile_pool(name="xtp", bufs=4) as xt_pool,
            tc.tile_pool(name="qkv_ps", bufs=2, space="PSUM") as qkv_ps,
            tc.tile_pool(name="stp_ps", bufs=1, space="PSUM") as stp_ps,
            tc.tile_pool(name="vt_ps", bufs=1, space="PSUM") as vt_ps,
            tc.tile_pool(name="av_ps", bufs=1, space="PSUM") as av_ps,
            tc.tile_pool(name="out_ps", bufs=3, space="PSUM") as out_ps,
            tc.tile_pool(name="qk_sb", bufs=2) as qk_sb,
            tc.tile_pool(name="t_sb", bufs=4) as t_sb,
            tc.tile_pool(name="h_sb", bufs=3) as h_sb,
            tc.tile_pool(name="v_sb", bufs=2) as v_sb_pool,
            tc.tile_pool(name="vt_sb", bufs=3) as vt_sb_pool,
            tc.tile_pool(name="est_sb", bufs=6) as est_sb,
            tc.tile_pool(name="av_sb", bufs=2) as av_sb_pool,
            tc.tile_pool(name="rec_sb", bufs=2) as rec_sb,
            tc.tile_pool(name="osb", bufs=3) as osb_pool,
        ):
            # ---- one-time loads: wq first (small, gates the first q-proj),
            # then x(0), rope tables, wk, x(1), wv, ones, ident, wo ----
            xt_tiles = {}
            w_sb = {}
            for name, src in (("wq", wqt),):
                t = singles.tile([128, NDC * HD], F16, name=name, tag=name)
                nc.sync.dma_start(out=t, in_=src)
                w_sb[name] = t
            # x(0) split into chunked DMAs so the first q-proj matmuls can
            # stream in right behind the transfers
            t = xt_pool.tile([128, NDC, TB], F16, name="xt", tag="xt")
            for lo, hi in ((0, 1), (1, 2)):
                nc.sync.dma_start(out=t[:, lo:hi, :], in_=xt[0, :, lo:hi, :])
            xt_tiles[0] = t
            for name, src in (("wk", wkt),):
                tw = singles.tile([128, NDC * HD], F16, name=name, tag=name)
                nc.sync.dma_start(out=tw, in_=src)
                w_sb[name] = tw
            for lo, hi in ((2, 4), (4, NDC)):
                nc.sync.dma_start(out=t[:, lo:hi, :], in_=xt[0, :, lo:hi, :])
            cos_sb = singles.tile([128, TB], F16, name="cosf", tag="cosf")
            nc.sync.dma_start(out=cos_sb, in_=cosf)
            sin_sb = singles.tile([128, TB], F16, name="sinf", tag="sinf")
            nc.sync.dma_start(out=sin_sb, in_=sinf)
            sin2_sb = singles.tile([128, TB], F16, name="sin2f", tag="sin2f")
            nc.sync.dma_start(out=sin2_sb, in_=sin2f)
            if NBLK > 1:
                t1 = xt_pool.tile([128, NDC, TB], F16, name="xt", tag="xt")
                nc.sync.dma_start(out=t1, in_=xt[1])
                xt_tiles[1] = t1
            for name, src in (("wv", wvt),):
                t = singles.tile([128, NDC * HD], F16, name=name, tag=name)
                nc.sync.dma_start(out=t, in_=src)
                w_sb[name] = t
            ones_sb = singles.tile([128, 1], F16, name="ones", tag="ones")
            nc.sync.dma_start(out=ones_sb, in_=onesd)
            id_sb = singles.tile([128, 128], F16, name="ident", tag="ident")
            nc.sync.dma_start(out=id_sb, in_=identd)
            wot_sb = singles.tile([HD, DIM], F16, name="wot", tag="wot")
            nc.sync.dma_start(out=wot_sb, in_=wot)

            # rolling per-block state
            st = {}

            def rope(i, which, ps):
                """Evacuate fp32 PSUM head to fp16 SBUF, rotate-half via two
                partition-shifted DVE muls, combine with cos mul + add."""
                qsb = qk_sb.tile([128, TB], F16, name=f"{which}sb",
                                 tag=f"{which}sb")
                nc.scalar.copy(qsb, ps)
                # rotate-half without cross-partition math ops: multiply by
                # the half-swapped sine table (same partitions), then two
                # partition-shifted single-input DVE copies (legal; only
                # two-input SBUF ops require equal base partitions)
                tf = t_sb.tile([128, TB], F16, name=f"tf_{which}",
                               tag=f"tf_{which}")
                nc.vector.tensor_mul(tf, qsb, sin2_sb)
                tt = t_sb.tile([128, TB], F16, name=f"t_{which}",
                               tag=f"t_{which}")
                nc.vector.tensor_copy(tt[0:64, :], tf[64:128, :])
                nc.vector.tensor_copy(tt[64:128, :], tf[0:64, :])
                h = h_sb.tile([128, TB], F16, name=f"{which}_h",
                              tag=f"{which}_h")
                nc.gpsimd.tensor_mul(h, qsb, cos_sb)
                nc.vector.tensor_add(h, h, tt)
                return h

            def proj(i, wname, out_tag, interleave=None):
                ps = qkv_ps.tile([128, TB], F32, name=out_tag, tag="qkv_ps")
                for dc in range(NDC):
                    nc.tensor.matmul(
                        ps,
                        lhsT=w_sb[wname][:, dc * HD:(dc + 1) * HD],
                        rhs=xt_tiles[i][:, dc, :],
                        start=(dc == 0),
                        stop=(dc == NDC - 1),
                    )
                return ps

            def out_mm(i, j):
                """One out-projection matmul (j = c*2+dc) for block i."""
                c, dc = divmod(j, 2)
                op = out_ps.tile([128, 512], F32, name="out_ps", tag="out_ps")
                nc.tensor.matmul(
                    op[0:CH, :],
                    lhsT=st[("av_sb", i)][:, c * CH:(c + 1) * CH],
                    rhs=wot_sb[:, dc * 512:(dc + 1) * 512],
                    start=True, stop=True,
                )
                return op

            def out_evac(i, j, op, eng):
                c, dc = divmod(j, 2)
                dst = st[("osb", i)][0:CH, c, dc * 512:(dc + 1) * 512]
                rec = st[("rec", i)][0:CH, c:c + 1]
                if eng == 0:
                    nc.scalar.mul(dst, op[0:CH, :], rec)
                elif eng == 1:
                    nc.vector.tensor_scalar_mul(dst, op[0:CH, :], rec)
                else:
                    nc.gpsimd.tensor_scalar_mul(dst, op[0:CH, :], rec)

            LAST = NBLK - 1
            for i in range(NBLK + 4):
                fwd_b = [i] if i < NBLK else []
                sc_b = [b for b in (i - 1,) if 0 <= b < NBLK]
                ve_b = list(sc_b)
                vt_b = list(ve_b)
                av_b = [b for b in (i - 2,) if 0 <= b < NBLK]
                out_b = [b for b in (i - 3,) if 0 <= b < NBLK]

                def score_chunk(b_, kc_):
                    k0, ksz = KCH[kc_]
                    sp = stp_ps.tile([128, TB], F32, name=f"stp{kc_}",
                                     tag="stp_ps")
                    q_h1, k_h1 = st[("q_h", b_)], st[("k_h", b_)]
                    for bi in range(BB):
                        nc.tensor.matmul(
                            sp[0:ksz, bi * S:(bi + 1) * S],
                            lhsT=k_h1[:, bi * S + k0:bi * S + k0 + ksz],
                            rhs=q_h1[:, bi * S:(bi + 1) * S],
                            start=True, stop=True,
                        )
                    e = est_sb.tile([128, TB], F16, name=f"est{kc_}",
                                    tag=f"est{kc_}")
                    nc.scalar.activation(
                        out=e[0:ksz, :], in_=sp[0:ksz, :],
                        func=mybir.ActivationFunctionType.Exp,
                        scale=EXP_SCALE,
                    )
                    return e

                def scores_part1(b):
                    mark(f"scores1({b})")
                    st[("est", b)] = [score_chunk(b, 0)]

                def scores_part2(b):
                    mark(f"scores2({b})")
                    st[("est", b)].append(score_chunk(b, 1))
                    st.pop(("q_h", b))
                    st.pop(("k_h", b))

                def v_evac(b):
                    mark(f"v-evac({b})")
                    v_prev = st.pop(("v_ps", b))
                    vsb = v_sb_pool.tile([128, BB * S], F16, name="v_sb",
                                         tag="v_sb")
                    nc.vector.tensor_copy(vsb, v_prev)
                    st[("v_sb", b)] = vsb

                def vt_pe(b):
                    mark(f"vT({b})")
                    vtp = vt_ps.tile([128, BB * VP], F16, name="vt_ps",
                                     tag="vt_ps")
                    vsb1 = st.pop(("v_sb", b))
                    for bi in range(BB):
                        for kc, (k0, ksz) in enumerate(KCH):
                            nc.tensor.transpose(
                                vtp[0:ksz, bi * VP + kc * 128:
                                    bi * VP + kc * 128 + 128],
                                vsb1[:, bi * S + k0:bi * S + k0 + ksz],
                                id_sb,
                            )
                    vt = vt_sb_pool.tile([128, BB, VP], F16, name="vt",
                                         tag="vt")
                    nc.vector.tensor_copy(vt[:, :, :], vtp)
                    st[("vt", b)] = vt

                def av_sums(b):
                    mark(f"AV+sums({b})")
                    av_t = av_ps.tile([128, TB + NCH], F32, name="av_ps",
                                      tag="av_ps")
                    est_prev = st.pop(("est", b))
                    vt_prev = st.pop(("vt", b))
                    for bi in range(BB):
                        for kc, (k0, ksz) in enumerate(KCH):
                            nc.tensor.matmul(
                                av_t[:, bi * S:(bi + 1) * S],
                                lhsT=vt_prev[0:ksz, bi,
                                             kc * 128:(kc + 1) * 128],
                                rhs=est_prev[kc][0:ksz, bi * S:(bi + 1) * S],
                                start=(kc == 0), stop=(kc == 1),
                            )
                    for c in range(NCH):
                        for kc, (k0, ksz) in enumerate(KCH):
                            nc.tensor.matmul(
                                av_t[0:CH, TB + c:TB + c + 1],
                                lhsT=est_prev[kc][0:ksz,
                                                  c * CH:(c + 1) * CH],
                                rhs=ones_sb[0:ksz, :],
                                start=(kc == 0), stop=(kc == 1),
                            )
                    avsb = av_sb_pool.tile([128, TB], F16, name="av_sb",
                                           tag="av_sb")
                    nc.scalar.copy(avsb, av_t[:, 0:TB])
                    st[("av_sb", b)] = avsb
                    rec = rec_sb.tile([128, NCH], F32, name="rec", tag="rec")
                    nc.vector.reciprocal(rec[0:CH, :],
                                         av_t[0:CH, TB:TB + NCH])
                    st[("rec", b)] = rec
                    st[("osb", b)] = osb_pool.tile(
                        [CH, NCH, DIM], F16, name="osb", tag="osb")

                # evac engines alternate ACT/DVE (GPSIMD cannot read
                # PSUM, so Pool is out for all evacuations)
                J_ENG = [0, 1, 0, 1, 0, 1, 0, 1]
                ops = {}

                def out_block(b, js):
                    mark(f"out{js[0]}{js[-1]}({b})")
                    for j in js:
                        ops[(b, j)] = out_mm(b, j)
                    for j in js:
                        out_evac(b, j, ops.pop((b, j)), J_ENG[j])

                def out_dma(b):
                    osb_t = st.pop(("osb", b))
                    if b == LAST:
                        # tail: per-chunk DMAs overlap the trailing evacs
                        for c in range(NCH):
                            nc.sync.dma_start(
                                out=outd[b, c], in_=osb_t[0:CH, c, :])
                    else:
                        nc.sync.dma_start(
                            out=outd[b].rearrange("c p d -> p c d"),
                            in_=osb_t[0:CH, :, :],
                        )

                # ---------- emission for iteration i ----------
                if 2 <= i + 2 < NBLK:
                    mark(f"x-dma({i + 2})")
                    t = xt_pool.tile([128, NDC, TB], F16, name="xt", tag="xt")
                    nc.sync.dma_start(out=t, in_=xt[i + 2])
                    xt_tiles[i + 2] = t

                for b in sc_b:
                    scores_part1(b)
                for b in ve_b:
                    v_evac(b)
                for b in av_b:
                    av_sums(b)
                for b in sc_b:
                    scores_part2(b)
                for b in vt_b:
                    vt_pe(b)

                for b in out_b:
                    out_block(b, [0, 1])
                if fwd_b:
                    mark(f"q-proj({i})")
                    q_ps = proj(i, "wq", "q_ps")
                    mark(f"rope-q({i})")
                    st[("q_h", i)] = rope(i, "q", q_ps)
                for b in out_b:
                    out_block(b, [2, 3])
                if fwd_b:
                    mark(f"k-proj+rope({i})")
                    k_ps = proj(i, "wk", "k_ps")
                    st[("k_h", i)] = rope(i, "k", k_ps)
                for b in out_b:
                    out_block(b, [4, 5])
                if fwd_b:
                    mark(f"v-proj({i})")
                    st[("v_ps", i)] = proj(i, "wv", "v_ps")
                for b in out_b:
                    out_block(b, [6, 7])
                    out_dma(b)

    _split_excess_waits(nc)
    return nc


_NC_CACHE = {}


def _get_nc():
    if "nc" not in _NC_CACHE:
        _NC_CACHE["nc"] = _build_nc()
    return _NC_CACHE["nc"]


def _host_prep(x, wq, wk, wv, wo):
    """Shared (non-x) device inputs + per-core x^T shards, all fp16."""
    perm = np.concatenate([np.arange(0, HD, 2), np.arange(1, HD, 2)])
    # weight layout [p, dc, h]: row d of w.T at (p=d%128, dc=d//128)
    wqt = np.ascontiguousarray(
        wq[perm].T.reshape(NDC, 128, HD).transpose(1, 0, 2)).astype(np.float16)
    wkt = np.ascontiguousarray(
        wk[perm].T.reshape(NDC, 128, HD).transpose(1, 0, 2)).astype(np.float16)
    wvt = np.ascontiguousarray(
        wv.T.reshape(NDC, 128, HD).transpose(1, 0, 2)).astype(np.float16)
    wot = np.ascontiguousarray(wo.T).astype(np.float16)

    inv_freq = 1.0 / BASE ** (np.arange(0, HD, 2, dtype=np.float64) / HD)
    ang = np.zeros((S, HD // 2), np.float64)
    ang[1:] = np.arange(S - 1, dtype=np.float64)[:, None] * inv_freq[None, :]
    cos_t = np.cos(ang).T  # [64, S]
    sin_t = np.sin(ang).T
    cosf = np.tile(np.concatenate([cos_t, cos_t], axis=0), (1, BB))
    # sign-folded: rotated = q*cosf + swap64(q)*sinf in one add
    sinf = np.tile(np.concatenate([-sin_t, sin_t], axis=0), (1, BB))

    sin2f = np.tile(np.concatenate([sin_t, -sin_t], axis=0), (1, BB))
    shared = {
        "wqt": wqt, "wkt": wkt, "wvt": wvt, "wot": wot,
        "cosf": np.ascontiguousarray(cosf).astype(np.float16),
        "sinf": np.ascontiguousarray(sinf).astype(np.float16),
        "sin2f": np.ascontiguousarray(sin2f).astype(np.float16),
        "onesd": np.ones((128, 1), np.float16),
        "identd": np.eye(128, dtype=np.float16),
    }
    xts = []
    for c in range(N_CORES):
        xc = x[c * BS:(c + 1) * BS].reshape(NBLK, TB, NDC, 128)
        # [blk, p, dc, t]: one contiguous 6.4KB run per partition per DMA
        xtc = np.ascontiguousarray(
            xc.transpose(0, 3, 2, 1)).astype(np.float16)
        xts.append(xtc)
    return shared, xts


def kernel(x, wq, wk, wv, wo):
    x = np.asarray(x, np.float32)
    wq = np.asarray(wq, np.float32)
    wk = np.asarray(wk, np.float32)
    wv = np.asarray(wv, np.float32)
    wo = np.asarray(wo, np.float32)

    shared, xts = _host_prep(x, wq, wk, wv, wo)
    in_maps = [dict(shared, xt=xts[c]) for c in range(N_CORES)]
    nc = _get_nc()
    res = run_bass_kernel_spmd(nc, in_maps, list(range(N_CORES)))
    outs = []
    for c in range(N_CORES):
        o = np.asarray(res.results[c]["outd"], np.float32)
        outs.append(o.reshape(BS, S, DIM))
    return np.concatenate(outs, axis=0)
